# revision 29
# baseline (speedup 1.0000x reference)
"""AutoCorrelation (FFT cross-correlation attention) kernel for 8 TRN2 NeuronCores.

Math (derived from the reference, all permutations resolved):
  for each (b, x):   # b batch, x head index
    Qcol[t, z] = queries[b, t, x, z]; Kcol[t, z] = keys[b, t, x, z]
    cor[tau, z] = (1/sqrt(E)) * irfft(rfft(Qcol, t) * conj(rfft(Kcol, t)))[tau]
    A = softmax over tau of cor                       # [tau, z]
    out[b, x, y, z] = sum_s values[b, x, y, s] * A[s, z]

The rfft/irfft over L=128 are computed as bf16 128x128 matmuls with packed
real-DFT constant matrices (f32 accumulation in PSUM).  The complex
cross-spectrum
  Pr = QrKr + QiKi ; Pi = QiKr - QrKi
is restructured so the vector engine does ONE fused elementwise product per
4-head block:
  [T1 | P12] = [Qhat | QU] . [Khat | KV]
where Qhat=[Qr;Qi], QU=[Qr+Qi;Qr-Qi], Khat=[Kr;Ki], KV=[Kr-Ki;Kr+Ki]
(the extra linear combinations are folded into the forward DFT constants),
and ALL additions are folded into the inverse-DFT matmul constants Wi1/Wi2
(accumulated in PSUM; softmax scale folded in as well).  On even blocks the
scalar engine evacuates both operands to SBUF so the product runs in the
DVE's 2x packed-bf16 mode; on odd blocks the product reads the q-side
straight from PSUM (1x) and only the k-side is copied -- this balances the
scalar and vector engines at ~70 us each.

Softmax over the partition axis: exp on the scalar engine, column sums via a
ones-matmul (which also broadcasts them across partitions for free),
reciprocal via the DVE's reciprocal_approx_fast, applied by the vector
engine during the PSUM->SBUF copy of the final matmul output.  The block
tails are emitted one block late (software pipelining) so the tensor engine
has fill work while the DVE computes the current block's product.

Sharding: batch b -> core b (B == 8 == n_cores), no communication.
Host-side prep: q/k are cast to bf16 (they only feed bf16 matmuls);
values is transposed to [s, x, y] (contraction axis onto partitions --
on-chip DMA transpose is 2-byte-only -- with the head axis kept inner so
group DMA rows stay 2-4 KiB contiguous) and cast to bf16.  The output comes
back as bf16 [y, x, z] per core and is transposed/upcast on the host.
Overall relative L2 error vs the f32 jax reference: ~5e-3 (gate 2e-2).
"""
import math

import numpy as np
import ml_dtypes

import concourse.bass as bass
import concourse.tile as tile
from concourse import bacc, mybir
from concourse.bass_utils import run_bass_kernel_spmd

B, L, H, E = 8, 128, 128, 128
N_CORES = 8
GROUP = 8                      # heads per DMA group
NBLK = 2                        # compute blocks per group (4 heads each)
SCALE = 1.0 / math.sqrt(E)

F32 = mybir.dt.float32
F32R = mybir.dt.float32r
BF16 = mybir.dt.bfloat16
AF = mybir.ActivationFunctionType


def build_dft_constants():
    """Constant matrices (float32).  Validated in proto.py against jax."""
    t = np.arange(L)[:, None]
    f = np.arange(65)[None, :]
    ang = 2.0 * np.pi * t * f / L
    C = np.cos(ang)               # [t, f]
    S = np.sin(ang)

    WfT = np.zeros((L, 128))
    WfT[:, 0:65] = C
    WfT[:, 65:128] = -S[:, 1:64]

    WuTq = np.zeros((L, 126))
    WuTq[:, 0:63] = C[:, 1:64] - S[:, 1:64]
    WuTq[:, 63:126] = C[:, 1:64] + S[:, 1:64]
    WuTk = np.zeros((L, 126))
    WuTk[:, 0:63] = C[:, 1:64] + S[:, 1:64]
    WuTk[:, 63:126] = C[:, 1:64] - S[:, 1:64]

    w = np.full(65, 2.0); w[0] = 1.0; w[64] = 1.0
    s_f = w * SCALE / L
    tau = np.arange(L)[None, :]
    fc = np.arange(65)[:, None]
    cos_ft = np.cos(2.0 * np.pi * fc * tau / L)
    sin_ft = np.sin(2.0 * np.pi * fc * tau / L)

    Wi1 = np.zeros((128, L))
    Wi1[0:65] = s_f[:, None] * cos_ft
    Wi1[65:128] = s_f[1:64, None] * cos_ft[1:64]

    Wi2 = np.zeros((126, L))
    Wi2[0:63] = -(s_f[1:64, None] / 2.0) * sin_ft[1:64]
    Wi2[63:126] = +(s_f[1:64, None] / 2.0) * sin_ft[1:64]

    f32 = np.float32
    return WfT.astype(f32), WuTq.astype(f32), WuTk.astype(f32), \
        Wi1.astype(f32), Wi2.astype(f32)


def _patch_act_tables():
    """Make Exp and Ln resolve to the combined natural_log_exp_and_others
    ACT table set (they live in separate sets by default, which costs a
    ~1.3us ACT_TABLE_LOAD on every alternation).  Positions are preserved
    because act_func_set_id is positional."""
    import concourse.bacc as bacc_mod
    if getattr(bacc_mod, "_act_tables_patched", False):
        return
    orig = bacc_mod.get_activation_tables

    def patched(arch):
        tabs = dict(orig(arch))
        for name in ("exp_and_others", "natural_log", "exp_and_friends"):
            if name in tabs and "natural_log_exp_and_others" in tabs:
                tabs[name] = set()
        return tabs

    bacc_mod.get_activation_tables = patched
    bacc_mod._act_tables_patched = True


def build_nc():
    """Build the per-core Bass program (identical on all 8 cores)."""
    _patch_act_tables()
    nc = bacc.Bacc(None, target_bir_lowering=False, debug=False)

    q_d = nc.dram_tensor("q", [L, H, E], BF16, kind="ExternalInput")
    k_d = nc.dram_tensor("k", [L, H, E], BF16, kind="ExternalInput")
    vt_d = nc.dram_tensor("vt", [L, H, L], BF16, kind="ExternalInput")
    cst_d = nc.dram_tensor("cst", [128, 636], BF16, kind="ExternalInput")
    out_d = nc.dram_tensor("out", [L, H, L], BF16, kind="ExternalOutput")

    with tile.TileContext(nc) as tc:
        with (
            tc.tile_pool(name="consts", bufs=1) as consts,
            tc.tile_pool(name="qk", bufs=2) as qkpool,
            tc.tile_pool(name="vg", bufs=2) as vgpool,
            tc.tile_pool(name="ks", bufs=3) as kspool,
            tc.tile_pool(name="prod", bufs=3) as prodpool,
            tc.tile_pool(name="eb", bufs=4) as ebpool,
            tc.tile_pool(name="rr", bufs=3) as rrpool,
            tc.tile_pool(name="ob", bufs=4) as obpool,
            tc.tile_pool(name="pq", bufs=1, space="PSUM") as pqpool,
            tc.tile_pool(name="pk", bufs=1, space="PSUM") as pkpool,
            tc.tile_pool(name="pmid", bufs=3, space="PSUM") as pmidpool,
            tc.tile_pool(name="pout", bufs=1, space="PSUM") as poutpool,
        ):
            cst_s = consts.tile([128, 636], BF16)
            nc.scalar.dma_start(out=cst_s[:], in_=cst_d[:])
            wft_s = cst_s[:, 0:128]
            wutq_s = cst_s[:, 128:254]
            wutk_s = cst_s[:, 254:380]
            wi1_s = cst_s[:, 380:508]
            wi2_s = cst_s[:126, 508:636]
            ones_s = consts.tile([128, 128], BF16)
            nc.gpsimd.memset(ones_s[:], 1.0)

            def emit_tail(st):
                """Tail of a block: irfft -> exp -> sums -> recip -> out-mms
                -> normalize -> store.  Emitted one block late so the PE has
                fill work while the DVE computes the current block's product."""
                t12, vg, c, x0, _bi = st
                pc = pmidpool.tile([128, 512], F32, tag="mid")
                nc.tensor.matmul(pc[:], wi1_s, t12[:, 0:512],
                                 start=True, stop=False)
                nc.tensor.matmul(pc[:], wi2_s, t12[:126, 512:1024],
                                 start=False, stop=True)
                eb = ebpool.tile([128, 512], BF16)
                nc.scalar.activation(eb[:], pc[:], AF.Exp)
                ps = pmidpool.tile([128, 512], F32, tag="mid")
                nc.tensor.matmul(ps[:], ones_s[:], eb[:], start=True, stop=True)
                rr = rrpool.tile([128, 512], F32)
                nc.vector.reciprocal_approx_fast(rr[:], ps[:])
                po = poutpool.tile([128, 512], F32)
                for hh in range(4):
                    nc.tensor.matmul(
                        po[:, hh * 128:(hh + 1) * 128],
                        vg[:, c + hh * 128: c + (hh + 1) * 128],
                        eb[:, hh * 128:(hh + 1) * 128],
                        start=True, stop=True,
                    )
                ob = obpool.tile([128, 512], BF16)
                nc.vector.tensor_mul(ob[:], po[:], rr[:])
                nc.sync.dma_start(
                    out=out_d[:, x0:x0 + 4, :],
                    in_=ob[:].rearrange("p (h z) -> p h z", h=4),
                )

            pending = []
            for g in range(H // GROUP):
                hsl = slice(g * GROUP, (g + 1) * GROUP)
                qg = qkpool.tile([128, GROUP * 128], BF16, tag="qg")
                nc.sync.dma_start(
                    out=qg[:].rearrange("p (h e) -> p h e", h=GROUP),
                    in_=q_d[:, hsl, :],
                )
                kg = qkpool.tile([128, GROUP * 128], BF16, tag="kg")
                nc.sync.dma_start(
                    out=kg[:].rearrange("p (h e) -> p h e", h=GROUP),
                    in_=k_d[:, hsl, :],
                )
                vg = vgpool.tile([128, GROUP * 128], BF16)
                nc.sync.dma_start(
                    out=vg[:].rearrange("p (h y) -> p h y", h=GROUP),
                    in_=vt_d[:, hsl, :],
                )

                for blk in range(NBLK):
                    c = blk * 512
                    qsl = qg[:, c:c + 512]
                    ksl = kg[:, c:c + 512]

                    pq = pqpool.tile([128, 1024], F32)
                    pk = pkpool.tile([128, 1024], F32)
                    ks = kspool.tile([128, 1024], BF16)
                    nc.tensor.matmul(pk[:, 0:512], wft_s, ksl,
                                     start=True, stop=True)
                    nc.scalar.copy(out=ks[:, 0:512], in_=pk[:, 0:512])
                    nc.tensor.matmul(pq[:, 0:512], wft_s, qsl,
                                     start=True, stop=True)
                    nc.tensor.matmul(pk[:126, 512:1024], wutk_s, ksl,
                                     start=True, stop=True)
                    nc.scalar.copy(out=ks[:126, 512:1024],
                                   in_=pk[:126, 512:1024])
                    nc.tensor.matmul(pq[:126, 512:1024], wutq_s, qsl,
                                     start=True, stop=True)

                    t12 = prodpool.tile([128, 1024], BF16)
                    if blk % 2 == 0:
                        # even blocks: evacuate the q-side too (alternating
                        # between scalar and vector engines) and run the
                        # SBUF x SBUF bf16 product on the otherwise-idle
                        # GPSIMD engine
                        qs = kspool.tile([128, 1024], BF16, tag="qs")
                        nc.scalar.copy(out=qs[:], in_=pq[:])
                        nc.vector.tensor_mul(t12[:], qs[:], ks[:])
                    else:
                        nc.vector.tensor_mul(t12[:], pq[:], ks[:])

                    pending.append((t12, vg, c, g * GROUP + blk * 4,
                                    g * NBLK + blk))
                    if len(pending) > 1:
                        emit_tail(pending.pop(0))

            while pending:
                emit_tail(pending.pop(0))
    nc.compile()
    return nc


_CACHE = {}


def _get_nc():
    if "nc" not in _CACHE:
        _CACHE["nc"] = build_nc()
    return _CACHE["nc"]


def make_in_maps(queries, keys, values):
    q = np.ascontiguousarray(np.asarray(queries, dtype=np.float32)).astype(
        ml_dtypes.bfloat16)
    k = np.ascontiguousarray(np.asarray(keys, dtype=np.float32)).astype(
        ml_dtypes.bfloat16)
    v = np.asarray(values, dtype=np.float32)
    # vt[b, s, x, y] = values[b, x, y, s]  (contraction axis s -> partitions,
    # head x kept adjacent to y so group DMA rows are 4 KiB contiguous)
    vt = np.ascontiguousarray(v.transpose(0, 3, 1, 2)).astype(ml_dtypes.bfloat16)
    WfT, WuTq, WuTk, Wi1, Wi2 = build_dft_constants()
    cst = np.zeros((128, 636), np.float32)
    cst[:, 0:128] = WfT
    cst[:, 128:254] = WuTq
    cst[:, 254:380] = WuTk
    cst[:, 380:508] = Wi1
    cst[:126, 508:636] = Wi2
    consts = {"cst": cst.astype(ml_dtypes.bfloat16)}
    return [
        {"q": q[b], "k": k[b], "vt": vt[b], **consts}
        for b in range(N_CORES)
    ]


def kernel(queries, keys, values, **run_kwargs):
    nc = _get_nc()
    in_maps = make_in_maps(queries, keys, values)
    try:
        res = run_bass_kernel_spmd(nc, in_maps, core_ids=list(range(N_CORES)),
                                   **run_kwargs)
    except Exception:
        # transient device hiccups (e.g. NRT_EXEC_UNIT_UNRECOVERABLE after a
        # wedged run) usually clear on retry
        import time as _time
        _time.sleep(5)
        res = run_bass_kernel_spmd(nc, in_maps, core_ids=list(range(N_CORES)),
                                   **run_kwargs)
    out = np.stack([np.asarray(res.results[b]["out"],
                               dtype=np.float32).transpose(1, 0, 2)
                    for b in range(N_CORES)])
    if run_kwargs:
        kernel.last_results = res
    return out


# revision 30
# speedup vs baseline: 1.1855x; 1.1855x over previous
"""AutoCorrelation (FFT cross-correlation attention) kernel for 8 TRN2 NeuronCores.

Math (derived from the reference, all permutations resolved):
  for each (b, x):   # b batch, x head index
    Qcol[t, z] = queries[b, t, x, z]; Kcol[t, z] = keys[b, t, x, z]
    cor[tau, z] = (1/sqrt(E)) * irfft(rfft(Qcol, t) * conj(rfft(Kcol, t)))[tau]
    A = softmax over tau of cor                       # [tau, z]
    out[b, x, y, z] = sum_s values[b, x, y, s] * A[s, z]

The rfft/irfft over L=128 are computed as bf16 128x128 matmuls with packed
real-DFT constant matrices (f32 accumulation in PSUM).  The complex
cross-spectrum
  Pr = QrKr + QiKi ; Pi = QiKr - QrKi
is restructured so the vector engine does ONE fused elementwise product per
4-head block:
  [T1 | P12] = [Qhat | QU] . [Khat | KV]
where Qhat=[Qr;Qi], QU=[Qr+Qi;Qr-Qi], Khat=[Kr;Ki], KV=[Kr-Ki;Kr+Ki]
(the extra linear combinations are folded into the forward DFT constants),
and ALL additions are folded into the inverse-DFT matmul constants Wi1/Wi2
(accumulated in PSUM; softmax scale folded in as well).  On even blocks the
scalar engine evacuates both operands to SBUF so the product runs in the
DVE's 2x packed-bf16 mode; on odd blocks the product reads the q-side
straight from PSUM (1x) and only the k-side is copied -- this balances the
scalar and vector engines at ~70 us each.

Softmax over the partition axis: exp on the scalar engine, column sums via a
ones-matmul (which also broadcasts them across partitions for free),
reciprocal via the DVE's reciprocal_approx_fast, applied by the vector
engine during the PSUM->SBUF copy of the final matmul output.  The block
tails are emitted one block late (software pipelining) so the tensor engine
has fill work while the DVE computes the current block's product.

Sharding: batch b -> core b (B == 8 == n_cores), no communication.
Host-side prep: q/k are cast to bf16 (they only feed bf16 matmuls);
values is transposed to [s, x, y] (contraction axis onto partitions --
on-chip DMA transpose is 2-byte-only -- with the head axis kept inner so
group DMA rows stay 2-4 KiB contiguous) and cast to bf16.  The output comes
back as bf16 [y, x, z] per core and is transposed/upcast on the host.
Overall relative L2 error vs the f32 jax reference: ~5e-3 (gate 2e-2).
"""
import math

import numpy as np
import ml_dtypes

import concourse.bass as bass
import concourse.tile as tile
from concourse import bacc, mybir
from concourse.bass_utils import run_bass_kernel_spmd

B, L, H, E = 8, 128, 128, 128
N_CORES = 8
GROUP = 8                      # heads per DMA group
NBLK = 2                        # compute blocks per group (4 heads each)
SCALE = 1.0 / math.sqrt(E)

F32 = mybir.dt.float32
F32R = mybir.dt.float32r
BF16 = mybir.dt.bfloat16
AF = mybir.ActivationFunctionType


def build_dft_constants():
    """Constant matrices (float32).  Validated in proto.py against jax."""
    t = np.arange(L)[:, None]
    f = np.arange(65)[None, :]
    ang = 2.0 * np.pi * t * f / L
    C = np.cos(ang)               # [t, f]
    S = np.sin(ang)

    WfT = np.zeros((L, 128))
    WfT[:, 0:65] = C
    WfT[:, 65:128] = -S[:, 1:64]

    WuTq = np.zeros((L, 126))
    WuTq[:, 0:63] = C[:, 1:64] - S[:, 1:64]
    WuTq[:, 63:126] = C[:, 1:64] + S[:, 1:64]
    WuTk = np.zeros((L, 126))
    WuTk[:, 0:63] = C[:, 1:64] + S[:, 1:64]
    WuTk[:, 63:126] = C[:, 1:64] - S[:, 1:64]

    w = np.full(65, 2.0); w[0] = 1.0; w[64] = 1.0
    s_f = w * SCALE / L
    tau = np.arange(L)[None, :]
    fc = np.arange(65)[:, None]
    cos_ft = np.cos(2.0 * np.pi * fc * tau / L)
    sin_ft = np.sin(2.0 * np.pi * fc * tau / L)

    Wi1 = np.zeros((128, L))
    Wi1[0:65] = s_f[:, None] * cos_ft
    Wi1[65:128] = s_f[1:64, None] * cos_ft[1:64]

    Wi2 = np.zeros((126, L))
    Wi2[0:63] = -(s_f[1:64, None] / 2.0) * sin_ft[1:64]
    Wi2[63:126] = +(s_f[1:64, None] / 2.0) * sin_ft[1:64]

    f32 = np.float32
    return WfT.astype(f32), WuTq.astype(f32), WuTk.astype(f32), \
        Wi1.astype(f32), Wi2.astype(f32)


def _patch_act_tables():
    """Make Exp and Ln resolve to the combined natural_log_exp_and_others
    ACT table set (they live in separate sets by default, which costs a
    ~1.3us ACT_TABLE_LOAD on every alternation).  Positions are preserved
    because act_func_set_id is positional."""
    import concourse.bacc as bacc_mod
    if getattr(bacc_mod, "_act_tables_patched", False):
        return
    orig = bacc_mod.get_activation_tables

    def patched(arch):
        tabs = dict(orig(arch))
        for name in ("exp_and_others", "natural_log", "exp_and_friends"):
            if name in tabs and "natural_log_exp_and_others" in tabs:
                tabs[name] = set()
        return tabs

    bacc_mod.get_activation_tables = patched
    bacc_mod._act_tables_patched = True


def build_nc():
    """Build the per-core Bass program (identical on all 8 cores)."""
    _patch_act_tables()
    nc = bacc.Bacc(None, target_bir_lowering=False, debug=False)

    q_d = nc.dram_tensor("q", [L, H, E], BF16, kind="ExternalInput")
    k_d = nc.dram_tensor("k", [L, H, E], BF16, kind="ExternalInput")
    vt_d = nc.dram_tensor("vt", [L, H, L], BF16, kind="ExternalInput")
    cst_d = nc.dram_tensor("cst", [128, 636], BF16, kind="ExternalInput")
    out_d = nc.dram_tensor("out", [L, H, L], BF16, kind="ExternalOutput")

    with tile.TileContext(nc) as tc:
        with (
            tc.tile_pool(name="consts", bufs=1) as consts,
            tc.tile_pool(name="qk", bufs=2) as qkpool,
            tc.tile_pool(name="vg", bufs=2) as vgpool,
            tc.tile_pool(name="ks", bufs=3) as kspool,
            tc.tile_pool(name="prod", bufs=3) as prodpool,
            tc.tile_pool(name="eb", bufs=4) as ebpool,
            tc.tile_pool(name="rr", bufs=3) as rrpool,
            tc.tile_pool(name="ob", bufs=4) as obpool,
            tc.tile_pool(name="pq", bufs=1, space="PSUM") as pqpool,
            tc.tile_pool(name="pk", bufs=1, space="PSUM") as pkpool,
            tc.tile_pool(name="pmid", bufs=3, space="PSUM") as pmidpool,
            tc.tile_pool(name="pout", bufs=1, space="PSUM") as poutpool,
        ):
            cst_s = consts.tile([128, 636], BF16)
            nc.scalar.dma_start(out=cst_s[:], in_=cst_d[:])
            wft_s = cst_s[:, 0:128]
            wutq_s = cst_s[:, 128:254]
            wutk_s = cst_s[:, 254:380]
            wi1_s = cst_s[:, 380:508]
            wi2_s = cst_s[:126, 508:636]
            ones_s = consts.tile([128, 128], BF16)
            nc.gpsimd.memset(ones_s[:], 1.0)

            def emit_tail(st):
                """Tail of a block: irfft -> exp -> sums -> recip -> out-mms
                -> normalize -> store.  Emitted one block late so the PE has
                fill work while the DVE computes the current block's product."""
                t12, vg, c, x0, _bi = st
                pc = pmidpool.tile([128, 512], F32, tag="mid")
                nc.tensor.matmul(pc[:], wi1_s, t12[:, 0:512],
                                 start=True, stop=False)
                nc.tensor.matmul(pc[:], wi2_s, t12[:126, 512:1024],
                                 start=False, stop=True)
                eb = ebpool.tile([128, 512], BF16)
                nc.scalar.activation(eb[:], pc[:], AF.Exp)
                ps = pmidpool.tile([128, 512], F32, tag="mid")
                nc.tensor.matmul(ps[:], ones_s[:], eb[:], start=True, stop=True)
                rr = rrpool.tile([128, 512], F32)
                nc.vector.reciprocal_approx_fast(rr[:], ps[:])
                po = poutpool.tile([128, 512], F32)
                for hh in range(4):
                    nc.tensor.matmul(
                        po[:, hh * 128:(hh + 1) * 128],
                        vg[:, c + hh * 128: c + (hh + 1) * 128],
                        eb[:, hh * 128:(hh + 1) * 128],
                        start=True, stop=True,
                    )
                ob = obpool.tile([128, 512], BF16)
                nc.vector.tensor_mul(ob[:], po[:], rr[:])
                nc.sync.dma_start(
                    out=out_d[:, x0:x0 + 4, :],
                    in_=ob[:].rearrange("p (h z) -> p h z", h=4),
                )

            pending = []
            for g in range(H // GROUP):
                hsl = slice(g * GROUP, (g + 1) * GROUP)
                qg = qkpool.tile([128, GROUP * 128], BF16, tag="qg")
                nc.sync.dma_start(
                    out=qg[:].rearrange("p (h e) -> p h e", h=GROUP),
                    in_=q_d[:, hsl, :],
                )
                kg = qkpool.tile([128, GROUP * 128], BF16, tag="kg")
                nc.sync.dma_start(
                    out=kg[:].rearrange("p (h e) -> p h e", h=GROUP),
                    in_=k_d[:, hsl, :],
                )
                vg = vgpool.tile([128, GROUP * 128], BF16)
                nc.sync.dma_start(
                    out=vg[:].rearrange("p (h y) -> p h y", h=GROUP),
                    in_=vt_d[:, hsl, :],
                )

                for blk in range(NBLK):
                    c = blk * 512
                    qsl = qg[:, c:c + 512]
                    ksl = kg[:, c:c + 512]

                    pq = pqpool.tile([128, 1024], F32)
                    pk = pkpool.tile([128, 1024], F32)
                    nc.tensor.matmul(pq[:, 0:512], wft_s, qsl,
                                     start=True, stop=True)
                    nc.tensor.matmul(pk[:, 0:512], wft_s, ksl,
                                     start=True, stop=True)
                    nc.tensor.matmul(pq[:126, 512:1024], wutq_s, qsl,
                                     start=True, stop=True)
                    nc.tensor.matmul(pk[:126, 512:1024], wutk_s, ksl,
                                     start=True, stop=True)

                    ks = kspool.tile([128, 1024], BF16)
                    nc.scalar.copy(out=ks[:], in_=pk[:])

                    t12 = prodpool.tile([128, 1024], BF16)
                    if blk % 2 == 0:
                        # even blocks: evacuate the q-side too (alternating
                        # between scalar and vector engines) and run the
                        # SBUF x SBUF bf16 product on the otherwise-idle
                        # GPSIMD engine
                        qs = kspool.tile([128, 1024], BF16, tag="qs")
                        nc.scalar.copy(out=qs[:], in_=pq[:])
                        nc.vector.tensor_mul(t12[:], qs[:], ks[:])
                    else:
                        nc.vector.tensor_mul(t12[:], pq[:], ks[:])

                    pending.append((t12, vg, c, g * GROUP + blk * 4,
                                    g * NBLK + blk))
                    if len(pending) > 1:
                        emit_tail(pending.pop(0))

            while pending:
                emit_tail(pending.pop(0))
    nc.compile()
    return nc


_CACHE = {}


def _get_nc():
    if "nc" not in _CACHE:
        _CACHE["nc"] = build_nc()
    return _CACHE["nc"]


def make_in_maps(queries, keys, values):
    q = np.ascontiguousarray(np.asarray(queries, dtype=np.float32)).astype(
        ml_dtypes.bfloat16)
    k = np.ascontiguousarray(np.asarray(keys, dtype=np.float32)).astype(
        ml_dtypes.bfloat16)
    v = np.asarray(values, dtype=np.float32)
    # vt[b, s, x, y] = values[b, x, y, s]  (contraction axis s -> partitions,
    # head x kept adjacent to y so group DMA rows are 4 KiB contiguous)
    vt = np.ascontiguousarray(v.transpose(0, 3, 1, 2)).astype(ml_dtypes.bfloat16)
    WfT, WuTq, WuTk, Wi1, Wi2 = build_dft_constants()
    cst = np.zeros((128, 636), np.float32)
    cst[:, 0:128] = WfT
    cst[:, 128:254] = WuTq
    cst[:, 254:380] = WuTk
    cst[:, 380:508] = Wi1
    cst[:126, 508:636] = Wi2
    consts = {"cst": cst.astype(ml_dtypes.bfloat16)}
    return [
        {"q": q[b], "k": k[b], "vt": vt[b], **consts}
        for b in range(N_CORES)
    ]


def kernel(queries, keys, values, **run_kwargs):
    nc = _get_nc()
    in_maps = make_in_maps(queries, keys, values)
    try:
        res = run_bass_kernel_spmd(nc, in_maps, core_ids=list(range(N_CORES)),
                                   **run_kwargs)
    except Exception:
        # transient device hiccups (e.g. NRT_EXEC_UNIT_UNRECOVERABLE after a
        # wedged run) usually clear on retry
        import time as _time
        _time.sleep(5)
        res = run_bass_kernel_spmd(nc, in_maps, core_ids=list(range(N_CORES)),
                                   **run_kwargs)
    out = np.stack([np.asarray(res.results[b]["out"],
                               dtype=np.float32).transpose(1, 0, 2)
                    for b in range(N_CORES)])
    if run_kwargs:
        kernel.last_results = res
    return out


# revision 31
# speedup vs baseline: 1.1996x; 1.0119x over previous
"""AutoCorrelation (FFT cross-correlation attention) kernel for 8 TRN2 NeuronCores.

Math (derived from the reference, all permutations resolved):
  for each (b, x):   # b batch, x head index
    Qcol[t, z] = queries[b, t, x, z]; Kcol[t, z] = keys[b, t, x, z]
    cor[tau, z] = (1/sqrt(E)) * irfft(rfft(Qcol, t) * conj(rfft(Kcol, t)))[tau]
    A = softmax over tau of cor                       # [tau, z]
    out[b, x, y, z] = sum_s values[b, x, y, s] * A[s, z]

The rfft/irfft over L=128 are computed as bf16 128x128 matmuls with packed
real-DFT constant matrices (f32 accumulation in PSUM).  The complex
cross-spectrum
  Pr = QrKr + QiKi ; Pi = QiKr - QrKi
is restructured so the vector engine does ONE fused elementwise product per
4-head block:
  [T1 | P12] = [Qhat | QU] . [Khat | KV]
where Qhat=[Qr;Qi], QU=[Qr+Qi;Qr-Qi], Khat=[Kr;Ki], KV=[Kr-Ki;Kr+Ki]
(the extra linear combinations are folded into the forward DFT constants),
and ALL additions are folded into the inverse-DFT matmul constants Wi1/Wi2
(accumulated in PSUM; softmax scale folded in as well).  On even blocks the
scalar engine evacuates both operands to SBUF so the product runs in the
DVE's 2x packed-bf16 mode; on odd blocks the product reads the q-side
straight from PSUM (1x) and only the k-side is copied -- this balances the
scalar and vector engines at ~70 us each.

Softmax over the partition axis: exp on the scalar engine, column sums via a
ones-matmul (which also broadcasts them across partitions for free),
reciprocal via the DVE's reciprocal_approx_fast, applied by the vector
engine during the PSUM->SBUF copy of the final matmul output.  The block
tails are emitted one block late (software pipelining) so the tensor engine
has fill work while the DVE computes the current block's product.

Sharding: batch b -> core b (B == 8 == n_cores), no communication.
Host-side prep: q/k are cast to bf16 (they only feed bf16 matmuls);
values is transposed to [s, x, y] (contraction axis onto partitions --
on-chip DMA transpose is 2-byte-only -- with the head axis kept inner so
group DMA rows stay 2-4 KiB contiguous) and cast to bf16.  The output comes
back as bf16 [y, x, z] per core and is transposed/upcast on the host.
Overall relative L2 error vs the f32 jax reference: ~5e-3 (gate 2e-2).
"""
import math

import numpy as np
import ml_dtypes

import concourse.bass as bass
import concourse.tile as tile
from concourse import bacc, mybir
from concourse.bass_utils import run_bass_kernel_spmd

B, L, H, E = 8, 128, 128, 128
N_CORES = 8
GROUP = 8                      # heads per DMA group
NBLK = 2                        # compute blocks per group (4 heads each)
SCALE = 1.0 / math.sqrt(E)

F32 = mybir.dt.float32
F32R = mybir.dt.float32r
BF16 = mybir.dt.bfloat16
AF = mybir.ActivationFunctionType


def build_dft_constants():
    """Constant matrices (float32).  Validated in proto.py against jax."""
    t = np.arange(L)[:, None]
    f = np.arange(65)[None, :]
    ang = 2.0 * np.pi * t * f / L
    C = np.cos(ang)               # [t, f]
    S = np.sin(ang)

    WfT = np.zeros((L, 128))
    WfT[:, 0:65] = C
    WfT[:, 65:128] = -S[:, 1:64]

    WuTq = np.zeros((L, 126))
    WuTq[:, 0:63] = C[:, 1:64] - S[:, 1:64]
    WuTq[:, 63:126] = C[:, 1:64] + S[:, 1:64]
    WuTk = np.zeros((L, 126))
    WuTk[:, 0:63] = C[:, 1:64] + S[:, 1:64]
    WuTk[:, 63:126] = C[:, 1:64] - S[:, 1:64]

    w = np.full(65, 2.0); w[0] = 1.0; w[64] = 1.0
    s_f = w * SCALE / L
    tau = np.arange(L)[None, :]
    fc = np.arange(65)[:, None]
    cos_ft = np.cos(2.0 * np.pi * fc * tau / L)
    sin_ft = np.sin(2.0 * np.pi * fc * tau / L)

    Wi1 = np.zeros((128, L))
    Wi1[0:65] = s_f[:, None] * cos_ft
    Wi1[65:128] = s_f[1:64, None] * cos_ft[1:64]

    Wi2 = np.zeros((126, L))
    Wi2[0:63] = -(s_f[1:64, None] / 2.0) * sin_ft[1:64]
    Wi2[63:126] = +(s_f[1:64, None] / 2.0) * sin_ft[1:64]

    f32 = np.float32
    return WfT.astype(f32), WuTq.astype(f32), WuTk.astype(f32), \
        Wi1.astype(f32), Wi2.astype(f32)


def _patch_act_tables():
    """Make Exp and Ln resolve to the combined natural_log_exp_and_others
    ACT table set (they live in separate sets by default, which costs a
    ~1.3us ACT_TABLE_LOAD on every alternation).  Positions are preserved
    because act_func_set_id is positional."""
    import concourse.bacc as bacc_mod
    if getattr(bacc_mod, "_act_tables_patched", False):
        return
    orig = bacc_mod.get_activation_tables

    def patched(arch):
        tabs = dict(orig(arch))
        for name in ("exp_and_others", "natural_log", "exp_and_friends"):
            if name in tabs and "natural_log_exp_and_others" in tabs:
                tabs[name] = set()
        return tabs

    bacc_mod.get_activation_tables = patched
    bacc_mod._act_tables_patched = True


def build_nc():
    """Build the per-core Bass program (identical on all 8 cores)."""
    _patch_act_tables()
    nc = bacc.Bacc(None, target_bir_lowering=False, debug=False)

    q_d = nc.dram_tensor("q", [L, H, E], BF16, kind="ExternalInput")
    k_d = nc.dram_tensor("k", [L, H, E], BF16, kind="ExternalInput")
    vt_d = nc.dram_tensor("vt", [L, H, L], BF16, kind="ExternalInput")
    cst_d = nc.dram_tensor("cst", [128, 636], BF16, kind="ExternalInput")
    out_d = nc.dram_tensor("out", [L, H, L], BF16, kind="ExternalOutput")

    with tile.TileContext(nc) as tc:
        with (
            tc.tile_pool(name="consts", bufs=1) as consts,
            tc.tile_pool(name="qk", bufs=2) as qkpool,
            tc.tile_pool(name="vg", bufs=2) as vgpool,
            tc.tile_pool(name="ks", bufs=3) as kspool,
            tc.tile_pool(name="prod", bufs=3) as prodpool,
            tc.tile_pool(name="eb", bufs=4) as ebpool,
            tc.tile_pool(name="rr", bufs=3) as rrpool,
            tc.tile_pool(name="ob", bufs=4) as obpool,
            tc.tile_pool(name="pq", bufs=1, space="PSUM") as pqpool,
            tc.tile_pool(name="pk", bufs=1, space="PSUM") as pkpool,
            tc.tile_pool(name="pmid", bufs=3, space="PSUM") as pmidpool,
            tc.tile_pool(name="pout", bufs=1, space="PSUM") as poutpool,
        ):
            cst_s = consts.tile([128, 636], BF16)
            nc.scalar.dma_start(out=cst_s[:], in_=cst_d[:])
            wft_s = cst_s[:, 0:128]
            wutq_s = cst_s[:, 128:254]
            wutk_s = cst_s[:, 254:380]
            wi1_s = cst_s[:, 380:508]
            wi2_s = cst_s[:126, 508:636]
            ones_s = consts.tile([128, 128], BF16)
            nc.gpsimd.memset(ones_s[:], 1.0)

            def emit_tail(st):
                """Tail of a block: irfft -> exp -> sums -> recip -> out-mms
                -> normalize -> store.  Emitted one block late so the PE has
                fill work while the DVE computes the current block's product."""
                t12, vg, c, x0, _bi = st
                pc = pmidpool.tile([128, 512], F32, tag="mid")
                nc.tensor.matmul(pc[:], wi1_s, t12[:, 0:512],
                                 start=True, stop=False)
                nc.tensor.matmul(pc[:], wi2_s, t12[:126, 512:1024],
                                 start=False, stop=True)
                eb = ebpool.tile([128, 512], BF16)
                nc.scalar.activation(eb[:], pc[:], AF.Exp)
                ps = pmidpool.tile([128, 512], F32, tag="mid")
                nc.tensor.matmul(ps[:], ones_s[:], eb[:], start=True, stop=True)
                rr = rrpool.tile([128, 512], F32)
                nc.vector.reciprocal_approx_fast(rr[:], ps[:])
                po = poutpool.tile([128, 512], F32)
                for hh in range(4):
                    nc.tensor.matmul(
                        po[:, hh * 128:(hh + 1) * 128],
                        vg[:, c + hh * 128: c + (hh + 1) * 128],
                        eb[:, hh * 128:(hh + 1) * 128],
                        start=True, stop=True,
                    )
                ob = obpool.tile([128, 512], BF16)
                nc.vector.tensor_mul(ob[:], po[:], rr[:])
                nc.sync.dma_start(
                    out=out_d[:, x0:x0 + 4, :],
                    in_=ob[:].rearrange("p (h z) -> p h z", h=4),
                )

            pending = []
            for g in range(H // GROUP):
                hsl = slice(g * GROUP, (g + 1) * GROUP)
                qg = qkpool.tile([128, GROUP * 128], BF16, tag="qg")
                nc.sync.dma_start(
                    out=qg[:].rearrange("p (h e) -> p h e", h=GROUP),
                    in_=q_d[:, hsl, :],
                )
                kg = qkpool.tile([128, GROUP * 128], BF16, tag="kg")
                nc.sync.dma_start(
                    out=kg[:].rearrange("p (h e) -> p h e", h=GROUP),
                    in_=k_d[:, hsl, :],
                )
                vg = vgpool.tile([128, GROUP * 128], BF16)
                nc.sync.dma_start(
                    out=vg[:].rearrange("p (h y) -> p h y", h=GROUP),
                    in_=vt_d[:, hsl, :],
                )

                for blk in range(NBLK):
                    c = blk * 512
                    qsl = qg[:, c:c + 512]
                    ksl = kg[:, c:c + 512]

                    pq = pqpool.tile([128, 1024], F32)
                    pk_h = pkpool.tile([128, 512], F32, tag="kh")
                    pk_v = pkpool.tile([128, 512], F32, tag="kv")
                    ks = kspool.tile([128, 1024], BF16)
                    nc.tensor.matmul(pk_h[:], wft_s, ksl,
                                     start=True, stop=True)
                    nc.tensor.matmul(pq[:, 0:512], wft_s, qsl,
                                     start=True, stop=True)
                    nc.scalar.copy(out=ks[:, 0:512], in_=pk_h[:])
                    nc.tensor.matmul(pk_v[:126, :], wutk_s, ksl,
                                     start=True, stop=True)
                    nc.tensor.matmul(pq[:126, 512:1024], wutq_s, qsl,
                                     start=True, stop=True)
                    nc.scalar.copy(out=ks[:126, 512:1024], in_=pk_v[:126, :])

                    t12 = prodpool.tile([128, 1024], BF16)
                    if blk % 2 == 0:
                        # even blocks: evacuate the q-side too (alternating
                        # between scalar and vector engines) and run the
                        # SBUF x SBUF bf16 product on the otherwise-idle
                        # GPSIMD engine
                        qs = kspool.tile([128, 1024], BF16, tag="qs")
                        nc.scalar.copy(out=qs[:], in_=pq[:])
                        nc.vector.tensor_mul(t12[:], qs[:], ks[:])
                    else:
                        nc.vector.tensor_mul(t12[:], pq[:], ks[:])

                    pending.append((t12, vg, c, g * GROUP + blk * 4,
                                    g * NBLK + blk))
                    if len(pending) > 1:
                        emit_tail(pending.pop(0))

            while pending:
                emit_tail(pending.pop(0))
    nc.compile()
    return nc


_CACHE = {}


def _get_nc():
    if "nc" not in _CACHE:
        _CACHE["nc"] = build_nc()
    return _CACHE["nc"]


def make_in_maps(queries, keys, values):
    q = np.ascontiguousarray(np.asarray(queries, dtype=np.float32)).astype(
        ml_dtypes.bfloat16)
    k = np.ascontiguousarray(np.asarray(keys, dtype=np.float32)).astype(
        ml_dtypes.bfloat16)
    v = np.asarray(values, dtype=np.float32)
    # vt[b, s, x, y] = values[b, x, y, s]  (contraction axis s -> partitions,
    # head x kept adjacent to y so group DMA rows are 4 KiB contiguous)
    vt = np.ascontiguousarray(v.transpose(0, 3, 1, 2)).astype(ml_dtypes.bfloat16)
    WfT, WuTq, WuTk, Wi1, Wi2 = build_dft_constants()
    cst = np.zeros((128, 636), np.float32)
    cst[:, 0:128] = WfT
    cst[:, 128:254] = WuTq
    cst[:, 254:380] = WuTk
    cst[:, 380:508] = Wi1
    cst[:126, 508:636] = Wi2
    consts = {"cst": cst.astype(ml_dtypes.bfloat16)}
    return [
        {"q": q[b], "k": k[b], "vt": vt[b], **consts}
        for b in range(N_CORES)
    ]


def kernel(queries, keys, values, **run_kwargs):
    nc = _get_nc()
    in_maps = make_in_maps(queries, keys, values)
    try:
        res = run_bass_kernel_spmd(nc, in_maps, core_ids=list(range(N_CORES)),
                                   **run_kwargs)
    except Exception:
        # transient device hiccups (e.g. NRT_EXEC_UNIT_UNRECOVERABLE after a
        # wedged run) usually clear on retry
        import time as _time
        _time.sleep(5)
        res = run_bass_kernel_spmd(nc, in_maps, core_ids=list(range(N_CORES)),
                                   **run_kwargs)
    out = np.stack([np.asarray(res.results[b]["out"],
                               dtype=np.float32).transpose(1, 0, 2)
                    for b in range(N_CORES)])
    if run_kwargs:
        kernel.last_results = res
    return out
